# revision 11
# baseline (speedup 1.0000x reference)
"""GCN (2-layer, PyG-style GCNConv) + edge MLP on 8 TRN2 NeuronCores.

Strategy:
- Nodes dst-sharded contiguously: core c owns dst rows [c*6250, (c+1)*6250).
- gcn_conv(x, W) is computed as (A @ x) @ W  (aggregate-then-project; exact
  reorder of the reference's (x @ W) gather since A is linear).
- Aggregation: edges sorted by dst, tiled into 128-dst windows and 128-edge
  chunks. Per chunk: indirect-DMA row gather of the source features + a
  selection matrix S[e,d] = norm[e]*(dstloc[e]==d) built on DVE, then a
  PSUM-accumulated matmul performs the scatter-add.
- Layer 2 needs p = relu(A@x@W1 + b1)@W2 rows from all cores -> AllGather.
- Edge MLP is data-parallel over edges, packed 2 edges/column so DMA runs at
  full 128-partition width; interleaved with aggregation windows so TensorE
  work overlaps gather DMA and the collective.

kernel(**inputs) takes the FULL inputs and returns (x_rec, e_rec).
"""

import numpy as np
import sys

sys.path.insert(0, "/opt/trn_rl_repo")

import concourse.bass as bass
import concourse.tile as tile
from concourse import bacc, mybir
from concourse.bass_utils import run_bass_kernel_spmd

P = 128
N_NODES = 50000
E_EDGES = 800000
D = 128          # node feature dim
H = 256          # hidden dim
DE = 64          # edge feature dim
NCORES = 8
NPC = N_NODES // NCORES          # 6250 nodes per core
WPC = (NPC + P - 1) // P         # 49 windows per core (last has 106 rows)
EPC = E_EDGES // NCORES          # 100000 edges per core (edge MLP shard)
E_TILE = 512                     # edge-MLP columns per tile (2 edges/col)
EMLP_TILES = (EPC // 2 + E_TILE - 1) // E_TILE   # 98
EMLP_COLS = EMLP_TILES * E_TILE                  # 50176

F32 = mybir.dt.float32
I32 = mybir.dt.int32


def _preprocess(edge_index):
    """Sort edges (plus self-loops) by dst, shard by dst range, window+chunk.

    Returns (cpws, per_core) where cpws[w] = chunks in window w (same for all
    cores; max over cores) and per_core[c] = (srcT, dstT, nrmT) arrays of
    shape [128, TC] (int32/f32/f32), chunk-major columns.
    """
    src = np.asarray(edge_index[0], dtype=np.int64)
    dst = np.asarray(edge_index[1], dtype=np.int64)
    loop = np.arange(N_NODES, dtype=np.int64)
    all_src = np.concatenate([src, loop])
    all_dst = np.concatenate([dst, loop])
    deg = np.bincount(all_dst, minlength=N_NODES).astype(np.float32)
    dinv = 1.0 / np.sqrt(deg)          # deg >= 1 (self loop)
    all_norm = (dinv[all_src] * dinv[all_dst]).astype(np.float32)

    order = np.argsort(all_dst, kind="stable")
    s_src = all_src[order].astype(np.int32)
    s_dst = all_dst[order]
    s_norm = all_norm[order]

    # window boundaries: for core c, window w covers dst
    # [c*NPC + w*P, min(c*NPC + (w+1)*P, (c+1)*NPC))
    bounds = np.empty((NCORES, WPC + 1), dtype=np.int64)
    for c in range(NCORES):
        lo = c * NPC
        hi = (c + 1) * NPC
        edges_w = np.minimum(lo + np.arange(WPC + 1) * P, hi)
        bounds[c] = np.searchsorted(s_dst, edges_w)

    counts = bounds[:, 1:] - bounds[:, :-1]             # [NCORES, WPC]
    cpws = np.maximum((counts + P - 1) // P, 1).max(axis=0)  # [WPC]
    TC = int(cpws.sum())

    per_core = []
    for c in range(NCORES):
        srcA = np.zeros((TC, P), dtype=np.int32)
        dstA = np.zeros((TC, P), dtype=np.float32)
        nrmA = np.zeros((TC, P), dtype=np.float32)
        ci = 0
        for w in range(WPC):
            lo, hi = bounds[c, w], bounds[c, w + 1]
            k = hi - lo
            cpw = int(cpws[w])
            flat_src = np.zeros(cpw * P, dtype=np.int32)
            flat_dst = np.zeros(cpw * P, dtype=np.float32)
            flat_nrm = np.zeros(cpw * P, dtype=np.float32)
            flat_src[:k] = s_src[lo:hi]
            flat_dst[:k] = (s_dst[lo:hi] - c * NPC - w * P).astype(np.float32)
            flat_nrm[:k] = s_norm[lo:hi]
            srcA[ci : ci + cpw] = flat_src.reshape(cpw, P)
            dstA[ci : ci + cpw] = flat_dst.reshape(cpw, P)
            nrmA[ci : ci + cpw] = flat_nrm.reshape(cpw, P)
            ci += cpw
        per_core.append(
            (
                np.ascontiguousarray(srcA.T),
                np.ascontiguousarray(dstA.T),
                np.ascontiguousarray(nrmA.T),
            )
        )
    return [int(v) for v in cpws], TC, per_core


def _build_program(cpws, TC):
    """Build the SPMD Bass program (identical across cores)."""
    nc = bacc.Bacc("TRN2", target_bir_lowering=False, debug=False,
                   num_devices=NCORES)

    x_d = nc.dram_tensor("x", [N_NODES, D], F32, kind="ExternalInput")
    srcT_d = nc.dram_tensor("srcT", [P, TC], I32, kind="ExternalInput")
    dstT_d = nc.dram_tensor("dstT", [P, TC], F32, kind="ExternalInput")
    nrmT_d = nc.dram_tensor("nrmT", [P, TC], F32, kind="ExternalInput")
    iota_d = nc.dram_tensor("iota", [P, P], F32, kind="ExternalInput")
    W1_d = nc.dram_tensor("W1", [D, H], F32, kind="ExternalInput")
    W2s_d = nc.dram_tensor("W2s", [P, H], F32, kind="ExternalInput")
    We1_d = nc.dram_tensor("We1", [P, H], F32, kind="ExternalInput")
    We2s_d = nc.dram_tensor("We2s", [P, P], F32, kind="ExternalInput")
    b1T_d = nc.dram_tensor("b1T", [P, 2], F32, kind="ExternalInput")
    be1T_d = nc.dram_tensor("be1T", [P, 2], F32, kind="ExternalInput")
    be2d_d = nc.dram_tensor("be2d", [P, 1], F32, kind="ExternalInput")
    xeP_d = nc.dram_tensor("xeP", [P, EMLP_COLS], F32, kind="ExternalInput")

    xrec_d = nc.dram_tensor("xrec", [NPC, D], F32, kind="ExternalOutput")
    erecP_d = nc.dram_tensor("erecP", [P, EMLP_COLS], F32, kind="ExternalOutput")

    p_loc_d = nc.dram_tensor("p_loc", [NPC, D], F32)
    p_full_d = nc.dram_tensor("p_full", [N_NODES, D], F32)

    with tile.TileContext(nc) as tc:
        with (
            tc.tile_pool(name="meta", bufs=1) as meta_tp,
            tc.tile_pool(name="msg", bufs=6) as msg_tp,
            tc.tile_pool(name="sel", bufs=6) as sel_tp,
            tc.tile_pool(name="wsb", bufs=2) as wsb_tp,
            tc.tile_pool(name="emlp", bufs=3) as emlp_tp,
            tc.tile_pool(name="agg_ps", bufs=2, space="PSUM") as aggps_tp,
            tc.tile_pool(name="h_ps", bufs=2, space="PSUM") as hps_tp,
            tc.tile_pool(name="p_ps", bufs=1, space="PSUM") as pps_tp,
            tc.tile_pool(name="e1_ps", bufs=2, space="PSUM") as e1ps_tp,
            tc.tile_pool(name="er_ps", bufs=1, space="PSUM") as erps_tp,
        ):
            # resident metadata + weights
            srcT = meta_tp.tile([P, TC], I32)
            dstT = meta_tp.tile([P, TC], F32)
            nrmT = meta_tp.tile([P, TC], F32)
            iota = meta_tp.tile([P, P], F32)
            W1 = meta_tp.tile([D, H], F32)
            W2s = meta_tp.tile([P, H], F32)
            We1 = meta_tp.tile([P, H], F32)   # We1 duplicated on both halves
            We2s = meta_tp.tile([P, P], F32)
            b1T = meta_tp.tile([P, 2], F32)
            be1T = meta_tp.tile([P, 2], F32)
            be2d = meta_tp.tile([P, 1], F32)
            nc.sync.dma_start(srcT[:], srcT_d.ap())
            nc.sync.dma_start(dstT[:], dstT_d.ap())
            nc.sync.dma_start(nrmT[:], nrmT_d.ap())
            nc.sync.dma_start(iota[:], iota_d.ap())
            nc.sync.dma_start(W1[:], W1_d.ap())
            nc.sync.dma_start(W2s[:], W2s_d.ap())
            nc.sync.dma_start(We1[:], We1_d.ap())
            nc.sync.dma_start(We2s[:], We2s_d.ap())
            nc.sync.dma_start(b1T[:], b1T_d.ap())
            nc.sync.dma_start(be1T[:], be1T_d.ap())
            nc.sync.dma_start(be2d[:], be2d_d.ap())

            def agg_window(w, ci0, cpw, table_d, out_is_dstmajor):
                """One 128-dst window of aggregation. Returns PSUM tile."""
                ps = aggps_tp.tile([P, P], F32, space="PSUM", tag="agg")
                for j in range(cpw):
                    ci = ci0 + j
                    msg = msg_tp.tile([P, D], F32, tag="msg")
                    nc.gpsimd.indirect_dma_start(
                        out=msg[:],
                        out_offset=None,
                        in_=table_d.ap(),
                        in_offset=bass.IndirectOffsetOnAxis(
                            ap=srcT[:, ci : ci + 1], axis=0
                        ),
                    )
                    S = sel_tp.tile([P, P], F32, tag="S")
                    nc.vector.tensor_scalar(
                        out=S[:],
                        in0=iota[:],
                        scalar1=dstT[:, ci : ci + 1],
                        scalar2=nrmT[:, ci : ci + 1],
                        op0=mybir.AluOpType.is_equal,
                        op1=mybir.AluOpType.mult,
                    )
                    if out_is_dstmajor:
                        nc.tensor.matmul(
                            out=ps[:], lhsT=S[:], rhs=msg[:],
                            start=(j == 0), stop=(j == cpw - 1),
                        )
                    else:
                        nc.tensor.matmul(
                            out=ps[:], lhsT=msg[:], rhs=S[:],
                            start=(j == 0), stop=(j == cpw - 1),
                        )
                return ps

            def layer1_window(w, ci0, cpw):
                # aggT [feat x dst] (orientation B)
                agg_ps = agg_window(w, ci0, cpw, x_d, out_is_dstmajor=False)
                aggT = wsb_tp.tile([P, P], F32, tag="aggT")
                nc.scalar.activation(
                    out=aggT[:], in_=agg_ps[:],
                    func=mybir.ActivationFunctionType.Copy,
                )
                # hT halves: [128h x 128dst] = W1half.T @ aggT
                h0ps = hps_tp.tile([P, P], F32, space="PSUM", tag="hps")
                h1ps = hps_tp.tile([P, P], F32, space="PSUM", tag="hps")
                nc.tensor.matmul(out=h0ps[:], lhsT=W1[:, 0:P], rhs=aggT[:],
                                 start=True, stop=True)
                nc.tensor.matmul(out=h1ps[:], lhsT=W1[:, P:H], rhs=aggT[:],
                                 start=True, stop=True)
                h0 = wsb_tp.tile([P, P], F32, tag="h0")
                h1 = wsb_tp.tile([P, P], F32, tag="h1")
                nc.scalar.activation(out=h0[:], in_=h0ps[:],
                                     func=mybir.ActivationFunctionType.Relu,
                                     bias=b1T[:, 0:1])
                nc.scalar.activation(out=h1[:], in_=h1ps[:],
                                     func=mybir.ActivationFunctionType.Relu,
                                     bias=b1T[:, 1:2])
                # p window [node x feat] = h.T @ W2 (accumulate over h halves)
                pps = pps_tp.tile([P, P], F32, space="PSUM", tag="pps")
                nc.tensor.matmul(out=pps[:], lhsT=h0[:], rhs=W2s[:, 0:P],
                                 start=True, stop=False)
                nc.tensor.matmul(out=pps[:], lhsT=h1[:], rhs=W2s[:, P:H],
                                 start=False, stop=True)
                psb = wsb_tp.tile([P, P], F32, tag="psb")
                nc.scalar.activation(out=psb[:], in_=pps[:],
                                     func=mybir.ActivationFunctionType.Copy)
                rows = min(P, NPC - w * P)
                nc.sync.dma_start(p_loc_d.ap()[w * P : w * P + rows, :],
                                  psb[0:rows, :])

            def layer2_window(w, ci0, cpw):
                # x_rec window [dst x feat] (orientation A)
                x_ps = agg_window(w, ci0, cpw, p_full_d, out_is_dstmajor=True)
                xr = wsb_tp.tile([P, P], F32, tag="xr")
                nc.scalar.activation(out=xr[:], in_=x_ps[:],
                                     func=mybir.ActivationFunctionType.Copy)
                rows = min(P, NPC - w * P)
                nc.sync.dma_start(xrec_d.ap()[w * P : w * P + rows, :],
                                  xr[0:rows, :])

            def emlp_tile(t):
                col0 = t * E_TILE
                xt = emlp_tp.tile([P, E_TILE], F32, tag="xt")
                nc.sync.dma_start(xt[:], xeP_d.ap()[:, col0 : col0 + E_TILE])
                er_ps = erps_tp.tile([P, E_TILE], F32, space="PSUM", tag="erps")
                for a in (0, 1):            # even/odd edge stream
                    e1sb = []
                    for hh in (0, 1):       # hidden halves
                        e1ps = e1ps_tp.tile([P, E_TILE], F32, space="PSUM",
                                            tag="e1ps")
                        nc.tensor.matmul(
                            out=e1ps[:],
                            lhsT=We1[a * DE : (a + 1) * DE, hh * P : (hh + 1) * P],
                            rhs=xt[a * DE : (a + 1) * DE, :],
                            start=True, stop=True,
                        )
                        sb = emlp_tp.tile([P, E_TILE], F32, tag="e1sb")
                        nc.scalar.activation(
                            out=sb[:], in_=e1ps[:],
                            func=mybir.ActivationFunctionType.Relu,
                            bias=be1T[:, hh : hh + 1],
                        )
                        e1sb.append(sb)
                    nc.tensor.matmul(
                        out=er_ps[a * DE : (a + 1) * DE, :],
                        lhsT=We2s[:, 0:DE],
                        rhs=e1sb[0][:],
                        start=True, stop=False,
                        tile_position=(0, a * DE),
                    )
                    nc.tensor.matmul(
                        out=er_ps[a * DE : (a + 1) * DE, :],
                        lhsT=We2s[:, DE : 2 * DE],
                        rhs=e1sb[1][:],
                        start=False, stop=True,
                        tile_position=(0, a * DE),
                    )
                er = emlp_tp.tile([P, E_TILE], F32, tag="er")
                nc.scalar.activation(out=er[:], in_=er_ps[:],
                                     func=mybir.ActivationFunctionType.Identity,
                                     bias=be2d[:])
                nc.sync.dma_start(erecP_d.ap()[:, col0 : col0 + E_TILE], er[:])

            # ---- phase A: layer-1 windows, interleaved with edge-MLP tiles
            ci0 = 0
            for w in range(WPC):
                layer1_window(w, ci0, cpws[w])
                ci0 += cpws[w]
                if w < EMLP_TILES // 2:
                    emlp_tile(w)

            # ---- AllGather p
            nc.gpsimd.collective_compute(
                "AllGather",
                mybir.AluOpType.bypass,
                replica_groups=[list(range(NCORES))],
                ins=[p_loc_d.ap()],
                outs=[p_full_d.ap()],
            )

            # ---- phase C: layer-2 windows, interleaved with remaining eMLP
            ci0 = 0
            for w in range(WPC):
                layer2_window(w, ci0, cpws[w])
                ci0 += cpws[w]
                if EMLP_TILES // 2 + w < EMLP_TILES:
                    emlp_tile(EMLP_TILES // 2 + w)
            for t in range(EMLP_TILES // 2 + WPC, EMLP_TILES):
                emlp_tile(t)

    nc.compile()
    return nc


_CACHE = {}


def _get_program(cpws, TC):
    key = tuple(cpws)
    if key not in _CACHE:
        _CACHE[key] = _build_program(cpws, TC)
    return _CACHE[key]


def kernel(x, edge_index, xe, W1, b1, W2, b2, We1, be1, We2, be2, **run_kwargs):
    out, _ = kernel_run(x, edge_index, xe, W1, b1, W2, b2, We1, be1, We2, be2,
                        **run_kwargs)
    return out


def kernel_run(x, edge_index, xe, W1, b1, W2, b2, We1, be1, We2, be2,
               **run_kwargs):
    x = np.asarray(x, dtype=np.float32)
    xe = np.asarray(xe, dtype=np.float32)
    W1 = np.asarray(W1, dtype=np.float32)
    W2 = np.asarray(W2, dtype=np.float32)
    We1 = np.asarray(We1, dtype=np.float32)
    We2 = np.asarray(We2, dtype=np.float32)
    b1 = np.asarray(b1, dtype=np.float32)
    b2 = np.asarray(b2, dtype=np.float32)
    be1 = np.asarray(be1, dtype=np.float32)
    be2 = np.asarray(be2, dtype=np.float32)

    cpws, TC, per_core = _preprocess(edge_index)
    nc = _get_program(cpws, TC)

    iota = np.tile(np.arange(P, dtype=np.float32), (P, 1))
    We1d = np.concatenate([We1, We1], axis=0)             # [128, 256] dup halves
    W2s = np.concatenate([W2[:P], W2[P:]], axis=1)        # [128, 256]
    We2s = np.concatenate([We2[:P], We2[P:]], axis=1)     # [128, 128]
    b1T = np.ascontiguousarray(b1.reshape(2, P).T)        # [128, 2]
    be1T = np.ascontiguousarray(be1.reshape(2, P).T)
    be2d = np.concatenate([be2, be2]).reshape(P, 1)

    in_maps = []
    for c in range(NCORES):
        srcT, dstT, nrmT = per_core[c]
        xe_c = xe[c * EPC : (c + 1) * EPC]
        xeP = np.zeros((P, EMLP_COLS), dtype=np.float32)
        xeP[:, : EPC // 2] = (
            xe_c.reshape(EPC // 2, 2, DE).transpose(1, 2, 0).reshape(P, EPC // 2)
        )
        in_maps.append({
            "x": x, "srcT": srcT, "dstT": dstT, "nrmT": nrmT, "iota": iota,
            "W1": W1, "W2s": W2s, "We1": We1d, "We2s": We2s,
            "b1T": b1T, "be1T": be1T, "be2d": be2d, "xeP": xeP,
        })

    res = run_bass_kernel_spmd(nc, in_maps, core_ids=list(range(NCORES)),
                               **run_kwargs)

    x_rec = np.concatenate([res.results[c]["xrec"] for c in range(NCORES)], axis=0)
    x_rec = x_rec + b2[None, :]
    e_rec = np.concatenate(
        [
            res.results[c]["erecP"]
            .reshape(2, DE, EMLP_COLS)[:, :, : EPC // 2]
            .transpose(2, 0, 1)
            .reshape(EPC, DE)
            for c in range(NCORES)
        ],
        axis=0,
    )
    return (x_rec.astype(np.float32), e_rec.astype(np.float32)), res


# revision 14
# speedup vs baseline: 1.1928x; 1.1928x over previous
"""GCN (2-layer, PyG-style GCNConv) + edge MLP on 8 TRN2 NeuronCores.

v2: bf16 data path + per-window dma_gather.

- Nodes dst-sharded contiguously: core c owns dst rows [c*6250, (c+1)*6250).
- gcn_conv(x, W) computed as (A @ x) @ W (exact linear reorder of reference).
- Aggregation: edges (+self loops) sorted by dst, tiled into 128-dst windows;
  within each window edges are split by src < 32768 (dma_gather indices are
  int16) into lo/hi groups, each padded to 128-edge chunks. One dma_gather per
  (window, half) pulls all source rows (bf16, 256B each). Per chunk a
  selection matrix S[e,d] = norm[e]*(dstloc[e]==d) is built on DVE and a
  PSUM-accumulated matmul performs the scatter-add.
- Layer 2 needs p = relu(A@x@W1 + b1)@W2 rows from all cores -> bf16
  AllGather into a per-core p_full gather table.
- Edge MLP data-parallel over edges, 2 edges packed per column; interleaved
  with aggregation windows so TensorE overlaps gather DMA and the collective.

kernel(**inputs) takes FULL inputs, returns (x_rec, e_rec) fp32.
"""

import numpy as np
import sys

sys.path.insert(0, "/opt/trn_rl_repo")

import ml_dtypes
import concourse.bass as bass
import concourse.tile as tile
from concourse import bacc, mybir
from concourse.bass_utils import run_bass_kernel_spmd

P = 128
N_NODES = 50000
E_EDGES = 800000
D = 128          # node feature dim
H = 256          # hidden dim
DE = 64          # edge feature dim
NCORES = 8
NPC = N_NODES // NCORES          # 6250 nodes per core
WPC = (NPC + P - 1) // P         # 49 windows per core (last has 106 rows)
EPC = E_EDGES // NCORES          # 100000 edges per core (edge MLP shard)
E_TILE = 512                     # edge-MLP columns per tile (2 edges/col)
EMLP_TILES = (EPC // 2 + E_TILE - 1) // E_TILE   # 98
EMLP_COLS = EMLP_TILES * E_TILE                  # 50176
HALF = 32768                     # int16 index limit -> table split

F32 = mybir.dt.float32
BF16 = mybir.dt.bfloat16
I16 = mybir.dt.int16

BF = ml_dtypes.bfloat16


def _preprocess(edge_index):
    """Sort edges+loops by dst; per core: window/halve/chunk + pack arrays.

    Returns:
      cpws_lo, cpws_hi: per-window chunk counts (shared across cores)
      per_core: list of dicts with idxP [128, 8*TC], dstT [128, TC] bf16,
                nrmT [128, TC] bf16  (TC = sum(cpws_lo + cpws_hi))
    """
    src = np.asarray(edge_index[0], dtype=np.int64)
    dst = np.asarray(edge_index[1], dtype=np.int64)
    loop = np.arange(N_NODES, dtype=np.int64)
    all_src = np.concatenate([src, loop])
    all_dst = np.concatenate([dst, loop])
    deg = np.bincount(all_dst, minlength=N_NODES).astype(np.float32)
    dinv = 1.0 / np.sqrt(deg)
    all_norm = (dinv[all_src] * dinv[all_dst]).astype(np.float32)

    order = np.argsort(all_dst, kind="stable")
    s_src = all_src[order].astype(np.int32)
    s_dst = all_dst[order]
    s_norm = all_norm[order]

    bounds = np.empty((NCORES, WPC + 1), dtype=np.int64)
    for c in range(NCORES):
        lo = c * NPC
        hi = (c + 1) * NPC
        edges_w = np.minimum(lo + np.arange(WPC + 1) * P, hi)
        bounds[c] = np.searchsorted(s_dst, edges_w)

    # split each (core, window) segment into lo/hi by src
    seg = {}
    nlo = np.zeros((NCORES, WPC), np.int64)
    nhi = np.zeros((NCORES, WPC), np.int64)
    for c in range(NCORES):
        for w in range(WPC):
            a, b = bounds[c, w], bounds[c, w + 1]
            ssrc = s_src[a:b]
            m = ssrc < HALF
            seg[c, w] = (ssrc[m], s_dst[a:b][m], s_norm[a:b][m],
                         ssrc[~m], s_dst[a:b][~m], s_norm[a:b][~m])
            nlo[c, w] = int(m.sum())
            nhi[c, w] = int((~m).sum())

    cpws_lo = np.maximum((nlo + P - 1) // P, 1).max(axis=0).astype(int)
    cpws_hi = ((nhi + P - 1) // P).max(axis=0).astype(int)   # may be 0
    TC = int(cpws_lo.sum() + cpws_hi.sum())

    per_core = []
    for c in range(NCORES):
        idxP = np.zeros((P, 8 * TC), dtype=np.int16)
        dstA = np.zeros((TC, P), dtype=np.float32)
        nrmA = np.zeros((TC, P), dtype=np.float32)
        ci = 0
        for w in range(WPC):
            slo, dlo, qlo, shi, dhi, qhi = seg[c, w]
            for (ss, dd, qq, cpw, base) in (
                (slo, dlo, qlo, int(cpws_lo[w]), 0),
                (shi - HALF, dhi, qhi, int(cpws_hi[w]), HALF),
            ):
                if cpw == 0:
                    continue
                n = cpw * P
                k = len(ss)
                fi = np.zeros(n, np.int16)
                fd = np.zeros(n, np.float32)
                fq = np.zeros(n, np.float32)
                fi[:k] = ss.astype(np.int16)
                fd[:k] = (dd - c * NPC) % P
                fq[:k] = qq
                # idx block: [16g + i%16, i//16] = fi[i], replicated g=0..7
                blk = fi.reshape(n // 16, 16).T          # [16, n/16]
                idxP[:, 8 * ci : 8 * ci + n // 16] = np.tile(blk, (8, 1))
                dstA[ci : ci + cpw] = fd.reshape(cpw, P)
                nrmA[ci : ci + cpw] = fq.reshape(cpw, P)
                ci += cpw
        assert ci == TC
        per_core.append({
            "idxP": idxP,
            "dstT": np.ascontiguousarray(dstA.T),
            "nrmT": np.ascontiguousarray(nrmA.T),
        })
    return [int(v) for v in cpws_lo], [int(v) for v in cpws_hi], TC, per_core


def _build_program(cpws_lo, cpws_hi, TC):
    nc = bacc.Bacc("TRN2", target_bir_lowering=False, debug=False,
                   num_devices=NCORES)

    x_d = nc.dram_tensor("x", [N_NODES, D], BF16, kind="ExternalInput")
    idxP_d = nc.dram_tensor("idxP", [P, 8 * TC], I16, kind="ExternalInput")
    dstT_d = nc.dram_tensor("dstT", [P, TC], F32, kind="ExternalInput")
    nrmT_d = nc.dram_tensor("nrmT", [P, TC], F32, kind="ExternalInput")
    iota_d = nc.dram_tensor("iota", [P, P], F32, kind="ExternalInput")
    W1_d = nc.dram_tensor("W1", [D, H], BF16, kind="ExternalInput")
    W2s_d = nc.dram_tensor("W2s", [P, H], BF16, kind="ExternalInput")
    We1_d = nc.dram_tensor("We1", [P, H], BF16, kind="ExternalInput")
    We2s_d = nc.dram_tensor("We2s", [P, P], BF16, kind="ExternalInput")
    b1T_d = nc.dram_tensor("b1T", [P, 2], F32, kind="ExternalInput")
    be1T_d = nc.dram_tensor("be1T", [P, 2], F32, kind="ExternalInput")
    be2d_d = nc.dram_tensor("be2d", [P, 1], F32, kind="ExternalInput")
    xeP_d = nc.dram_tensor("xeP", [P, EMLP_COLS], BF16, kind="ExternalInput")

    xrec_d = nc.dram_tensor("xrec", [NPC, D], F32, kind="ExternalOutput")
    erecP_d = nc.dram_tensor("erecP", [P, EMLP_COLS], F32, kind="ExternalOutput")

    p_loc_d = nc.dram_tensor("p_loc", [NPC, D], BF16)
    p_full_d = nc.dram_tensor("p_full", [N_NODES, D], BF16)

    with tile.TileContext(nc) as tc:
        with (
            tc.tile_pool(name="meta", bufs=1) as meta_tp,
            tc.tile_pool(name="msg", bufs=3) as msg_tp,
            tc.tile_pool(name="sel", bufs=6) as sel_tp,
            tc.tile_pool(name="wsb", bufs=2) as wsb_tp,
            tc.tile_pool(name="emlp", bufs=3) as emlp_tp,
            tc.tile_pool(name="agg_ps", bufs=2, space="PSUM") as aggps_tp,
            tc.tile_pool(name="h_ps", bufs=2, space="PSUM") as hps_tp,
            tc.tile_pool(name="p_ps", bufs=1, space="PSUM") as pps_tp,
            tc.tile_pool(name="e1_ps", bufs=2, space="PSUM") as e1ps_tp,
            tc.tile_pool(name="er_ps", bufs=1, space="PSUM") as erps_tp,
        ):
            idxP = meta_tp.tile([P, 8 * TC], I16)
            dstT = meta_tp.tile([P, TC], F32)
            nrmT = meta_tp.tile([P, TC], F32)
            iota = meta_tp.tile([P, P], F32)
            W1 = meta_tp.tile([D, H], BF16)
            W2s = meta_tp.tile([P, H], BF16)
            We1 = meta_tp.tile([P, H], BF16)
            We2s = meta_tp.tile([P, P], BF16)
            b1T = meta_tp.tile([P, 2], F32)
            be1T = meta_tp.tile([P, 2], F32)
            be2d = meta_tp.tile([P, 1], F32)
            for sb, dr in ((idxP, idxP_d), (dstT, dstT_d), (nrmT, nrmT_d),
                           (iota, iota_d), (W1, W1_d), (W2s, W2s_d),
                           (We1, We1_d), (We2s, We2s_d), (b1T, b1T_d),
                           (be1T, be1T_d), (be2d, be2d_d)):
                nc.sync.dma_start(sb[:], dr.ap())

            def agg_window(w, ci0, table_d, out_is_dstmajor):
                """One 128-dst window: 1-2 dma_gathers + chunk matmuls."""
                cl, ch = cpws_lo[w], cpws_hi[w]
                cpw = cl + ch
                ps = aggps_tp.tile([P, P], F32, space="PSUM", tag="agg")
                msgw = msg_tp.tile([P, cpw, D], BF16, tag="msg")
                GMAX = 8   # dma_gather fails above 1024 indices
                for (coff0, cn0, row0) in ((0, cl, 0), (cl, ch, HALF)):
                    for sub in range(0, cn0, GMAX):
                        coff = coff0 + sub
                        cn = min(GMAX, cn0 - sub)
                        nidx = cn * P
                        nc.gpsimd.dma_gather(
                            out_ap=msgw[:, coff : coff + cn, :],
                            in_ap=table_d.ap()[row0:, :],
                            idxs_ap=idxP[:, 8 * (ci0 + coff) : 8 * (ci0 + coff) + nidx // 16],
                            num_idxs=nidx,
                            num_idxs_reg=nidx,
                            elem_size=D,
                        )
                for j in range(cpw):
                    ci = ci0 + j
                    S = sel_tp.tile([P, P], BF16, tag="S")
                    nc.vector.tensor_scalar(
                        out=S[:],
                        in0=iota[:],
                        scalar1=dstT[:, ci : ci + 1],
                        scalar2=nrmT[:, ci : ci + 1],
                        op0=mybir.AluOpType.is_equal,
                        op1=mybir.AluOpType.mult,
                    )
                    if out_is_dstmajor:
                        nc.tensor.matmul(out=ps[:], lhsT=S[:],
                                         rhs=msgw[:, j, :],
                                         start=(j == 0), stop=(j == cpw - 1))
                    else:
                        nc.tensor.matmul(out=ps[:], lhsT=msgw[:, j, :],
                                         rhs=S[:],
                                         start=(j == 0), stop=(j == cpw - 1))
                return ps

            def layer1_window(w, ci0):
                agg_ps = agg_window(w, ci0, x_d, out_is_dstmajor=False)
                aggT = wsb_tp.tile([P, P], BF16, tag="aggT")
                nc.scalar.activation(out=aggT[:], in_=agg_ps[:],
                                     func=mybir.ActivationFunctionType.Copy)
                h0ps = hps_tp.tile([P, P], F32, space="PSUM", tag="hps")
                h1ps = hps_tp.tile([P, P], F32, space="PSUM", tag="hps")
                nc.tensor.matmul(out=h0ps[:], lhsT=W1[:, 0:P], rhs=aggT[:],
                                 start=True, stop=True)
                nc.tensor.matmul(out=h1ps[:], lhsT=W1[:, P:H], rhs=aggT[:],
                                 start=True, stop=True)
                h0 = wsb_tp.tile([P, P], BF16, tag="h0")
                h1 = wsb_tp.tile([P, P], BF16, tag="h1")
                nc.scalar.activation(out=h0[:], in_=h0ps[:],
                                     func=mybir.ActivationFunctionType.Relu,
                                     bias=b1T[:, 0:1])
                nc.scalar.activation(out=h1[:], in_=h1ps[:],
                                     func=mybir.ActivationFunctionType.Relu,
                                     bias=b1T[:, 1:2])
                pps = pps_tp.tile([P, P], F32, space="PSUM", tag="pps")
                nc.tensor.matmul(out=pps[:], lhsT=h0[:], rhs=W2s[:, 0:P],
                                 start=True, stop=False)
                nc.tensor.matmul(out=pps[:], lhsT=h1[:], rhs=W2s[:, P:H],
                                 start=False, stop=True)
                psb = wsb_tp.tile([P, P], BF16, tag="psb")
                nc.scalar.activation(out=psb[:], in_=pps[:],
                                     func=mybir.ActivationFunctionType.Copy)
                rows = min(P, NPC - w * P)
                nc.sync.dma_start(p_loc_d.ap()[w * P : w * P + rows, :],
                                  psb[0:rows, :])

            def layer2_window(w, ci0):
                x_ps = agg_window(w, ci0, p_full_d, out_is_dstmajor=True)
                xr = wsb_tp.tile([P, P], F32, tag="xr")
                nc.scalar.activation(out=xr[:], in_=x_ps[:],
                                     func=mybir.ActivationFunctionType.Copy)
                rows = min(P, NPC - w * P)
                nc.sync.dma_start(xrec_d.ap()[w * P : w * P + rows, :],
                                  xr[0:rows, :])

            def emlp_tile(t):
                col0 = t * E_TILE
                xt = emlp_tp.tile([P, E_TILE], BF16, tag="xt")
                nc.sync.dma_start(xt[:], xeP_d.ap()[:, col0 : col0 + E_TILE])
                er_ps = erps_tp.tile([P, E_TILE], F32, space="PSUM", tag="erps")
                for a in (0, 1):
                    e1sb = []
                    for hh in (0, 1):
                        e1ps = e1ps_tp.tile([P, E_TILE], F32, space="PSUM",
                                            tag="e1ps")
                        nc.tensor.matmul(
                            out=e1ps[:],
                            lhsT=We1[a * DE : (a + 1) * DE, hh * P : (hh + 1) * P],
                            rhs=xt[a * DE : (a + 1) * DE, :],
                            start=True, stop=True,
                        )
                        sb = emlp_tp.tile([P, E_TILE], BF16, tag="e1sb")
                        nc.scalar.activation(
                            out=sb[:], in_=e1ps[:],
                            func=mybir.ActivationFunctionType.Relu,
                            bias=be1T[:, hh : hh + 1],
                        )
                        e1sb.append(sb)
                    nc.tensor.matmul(
                        out=er_ps[a * DE : (a + 1) * DE, :],
                        lhsT=We2s[:, 0:DE],
                        rhs=e1sb[0][:],
                        start=True, stop=False,
                        tile_position=(0, a * DE),
                    )
                    nc.tensor.matmul(
                        out=er_ps[a * DE : (a + 1) * DE, :],
                        lhsT=We2s[:, DE : 2 * DE],
                        rhs=e1sb[1][:],
                        start=False, stop=True,
                        tile_position=(0, a * DE),
                    )
                er = emlp_tp.tile([P, E_TILE], F32, tag="er")
                nc.scalar.activation(out=er[:], in_=er_ps[:],
                                     func=mybir.ActivationFunctionType.Identity,
                                     bias=be2d[:])
                nc.sync.dma_start(erecP_d.ap()[:, col0 : col0 + E_TILE], er[:])

            # ---- phase A: layer-1 windows interleaved with edge-MLP tiles
            ci0 = 0
            for w in range(WPC):
                layer1_window(w, ci0)
                ci0 += cpws_lo[w] + cpws_hi[w]
                if w < EMLP_TILES // 2:
                    emlp_tile(w)

            # ---- AllGather p (bf16)
            nc.gpsimd.collective_compute(
                "AllGather",
                mybir.AluOpType.bypass,
                replica_groups=[list(range(NCORES))],
                ins=[p_loc_d.ap()],
                outs=[p_full_d.ap()],
            )

            # ---- phase C: layer-2 windows + remaining eMLP
            ci0 = 0
            for w in range(WPC):
                layer2_window(w, ci0)
                ci0 += cpws_lo[w] + cpws_hi[w]
                if EMLP_TILES // 2 + w < EMLP_TILES:
                    emlp_tile(EMLP_TILES // 2 + w)
            for t in range(EMLP_TILES // 2 + WPC, EMLP_TILES):
                emlp_tile(t)

    nc.compile()
    return nc


_CACHE = {}


def _get_program(cpws_lo, cpws_hi, TC):
    key = (tuple(cpws_lo), tuple(cpws_hi))
    if key not in _CACHE:
        _CACHE[key] = _build_program(cpws_lo, cpws_hi, TC)
    return _CACHE[key]


def kernel(x, edge_index, xe, W1, b1, W2, b2, We1, be1, We2, be2, **run_kwargs):
    out, _ = kernel_run(x, edge_index, xe, W1, b1, W2, b2, We1, be1, We2, be2,
                        **run_kwargs)
    return out


def kernel_run(x, edge_index, xe, W1, b1, W2, b2, We1, be1, We2, be2,
               **run_kwargs):
    x = np.asarray(x, dtype=np.float32)
    xe = np.asarray(xe, dtype=np.float32)
    W1 = np.asarray(W1, dtype=np.float32)
    W2 = np.asarray(W2, dtype=np.float32)
    We1 = np.asarray(We1, dtype=np.float32)
    We2 = np.asarray(We2, dtype=np.float32)
    b1 = np.asarray(b1, dtype=np.float32)
    b2 = np.asarray(b2, dtype=np.float32)
    be1 = np.asarray(be1, dtype=np.float32)
    be2 = np.asarray(be2, dtype=np.float32)

    cpws_lo, cpws_hi, TC, per_core = _preprocess(edge_index)
    nc = _get_program(cpws_lo, cpws_hi, TC)

    x_bf = x.astype(BF)
    iota = np.tile(np.arange(P, dtype=np.float32), (P, 1))
    We1d = np.concatenate([We1, We1], axis=0).astype(BF)     # [128, 256]
    W2s = np.concatenate([W2[:P], W2[P:]], axis=1).astype(BF)
    We2s = np.concatenate([We2[:P], We2[P:]], axis=1).astype(BF)
    W1b = W1.astype(BF)
    b1T = np.ascontiguousarray(b1.reshape(2, P).T)
    be1T = np.ascontiguousarray(be1.reshape(2, P).T)
    be2d = np.concatenate([be2, be2]).reshape(P, 1)

    in_maps = []
    for c in range(NCORES):
        pc = per_core[c]
        xe_c = xe[c * EPC : (c + 1) * EPC]
        xeP = np.zeros((P, EMLP_COLS), dtype=BF)
        xeP[:, : EPC // 2] = (
            xe_c.reshape(EPC // 2, 2, DE).transpose(1, 2, 0).reshape(P, EPC // 2)
        ).astype(BF)
        in_maps.append({
            "x": x_bf, "idxP": pc["idxP"], "dstT": pc["dstT"],
            "nrmT": pc["nrmT"], "iota": iota,
            "W1": W1b, "W2s": W2s, "We1": We1d, "We2s": We2s,
            "b1T": b1T, "be1T": be1T, "be2d": be2d, "xeP": xeP,
        })

    res = run_bass_kernel_spmd(nc, in_maps, core_ids=list(range(NCORES)),
                               **run_kwargs)

    x_rec = np.concatenate([res.results[c]["xrec"] for c in range(NCORES)], axis=0)
    x_rec = x_rec + b2[None, :]
    e_rec = np.concatenate(
        [
            res.results[c]["erecP"]
            .reshape(2, DE, EMLP_COLS)[:, :, : EPC // 2]
            .transpose(2, 0, 1)
            .reshape(EPC, DE)
            for c in range(NCORES)
        ],
        axis=0,
    )
    return (x_rec.astype(np.float32), e_rec.astype(np.float32)), res


# revision 15
# speedup vs baseline: 1.2109x; 1.0152x over previous
"""GCN (2-layer, PyG-style GCNConv) + edge MLP on 8 TRN2 NeuronCores.

v2: bf16 data path + per-window dma_gather.

- Nodes dst-sharded contiguously: core c owns dst rows [c*6250, (c+1)*6250).
- gcn_conv(x, W) computed as (A @ x) @ W (exact linear reorder of reference).
- Aggregation: edges (+self loops) sorted by dst, tiled into 128-dst windows;
  within each window edges are split by src < 32768 (dma_gather indices are
  int16) into lo/hi groups, each padded to 128-edge chunks. One dma_gather per
  (window, half) pulls all source rows (bf16, 256B each). Per chunk a
  selection matrix S[e,d] = norm[e]*(dstloc[e]==d) is built on DVE and a
  PSUM-accumulated matmul performs the scatter-add.
- Layer 2 needs p = relu(A@x@W1 + b1)@W2 rows from all cores -> bf16
  AllGather into a per-core p_full gather table.
- Edge MLP data-parallel over edges, 2 edges packed per column; interleaved
  with aggregation windows so TensorE overlaps gather DMA and the collective.

kernel(**inputs) takes FULL inputs, returns (x_rec, e_rec) fp32.
"""

import numpy as np
import sys

sys.path.insert(0, "/opt/trn_rl_repo")

import ml_dtypes
import concourse.bass as bass
import concourse.tile as tile
from concourse import bacc, mybir
from concourse.bass_utils import run_bass_kernel_spmd

P = 128
N_NODES = 50000
E_EDGES = 800000
D = 128          # node feature dim
H = 256          # hidden dim
DE = 64          # edge feature dim
NCORES = 8
NPC = N_NODES // NCORES          # 6250 nodes per core
WPC = (NPC + P - 1) // P         # 49 windows per core (last has 106 rows)
EPC = E_EDGES // NCORES          # 100000 edges per core (edge MLP shard)
E_TILE = 512                     # edge-MLP columns per tile (2 edges/col)
EMLP_TILES = (EPC // 2 + E_TILE - 1) // E_TILE   # 98
EMLP_COLS = EMLP_TILES * E_TILE                  # 50176
HALF = 32768                     # int16 index limit -> table split

F32 = mybir.dt.float32
BF16 = mybir.dt.bfloat16
I16 = mybir.dt.int16

BF = ml_dtypes.bfloat16


def _preprocess(edge_index):
    """Sort edges+loops by dst; per core: window/halve/chunk + pack arrays.

    Returns:
      cpws_lo, cpws_hi: per-window chunk counts (shared across cores)
      per_core: list of dicts with idxP [128, 8*TC], dstT [128, TC] bf16,
                nrmT [128, TC] bf16  (TC = sum(cpws_lo + cpws_hi))
    """
    src = np.asarray(edge_index[0], dtype=np.int64)
    dst = np.asarray(edge_index[1], dtype=np.int64)
    loop = np.arange(N_NODES, dtype=np.int64)
    all_src = np.concatenate([src, loop])
    all_dst = np.concatenate([dst, loop])
    deg = np.bincount(all_dst, minlength=N_NODES).astype(np.float32)
    dinv = 1.0 / np.sqrt(deg)
    all_norm = (dinv[all_src] * dinv[all_dst]).astype(np.float32)

    order = np.argsort(all_dst, kind="stable")
    s_src = all_src[order].astype(np.int32)
    s_dst = all_dst[order]
    s_norm = all_norm[order]

    bounds = np.empty((NCORES, WPC + 1), dtype=np.int64)
    for c in range(NCORES):
        lo = c * NPC
        hi = (c + 1) * NPC
        edges_w = np.minimum(lo + np.arange(WPC + 1) * P, hi)
        bounds[c] = np.searchsorted(s_dst, edges_w)

    # split each (core, window) segment into lo/hi by src
    seg = {}
    nlo = np.zeros((NCORES, WPC), np.int64)
    nhi = np.zeros((NCORES, WPC), np.int64)
    for c in range(NCORES):
        for w in range(WPC):
            a, b = bounds[c, w], bounds[c, w + 1]
            ssrc = s_src[a:b]
            m = ssrc < HALF
            seg[c, w] = (ssrc[m], s_dst[a:b][m], s_norm[a:b][m],
                         ssrc[~m], s_dst[a:b][~m], s_norm[a:b][~m])
            nlo[c, w] = int(m.sum())
            nhi[c, w] = int((~m).sum())

    cpws_lo = np.maximum((nlo + P - 1) // P, 1).max(axis=0).astype(int)
    cpws_hi = ((nhi + P - 1) // P).max(axis=0).astype(int)   # may be 0
    TC = int(cpws_lo.sum() + cpws_hi.sum())

    per_core = []
    for c in range(NCORES):
        idxP = np.zeros((P, 8 * TC), dtype=np.int16)
        Splane = np.zeros((P, TC * P), dtype=np.float32)
        ci = 0
        for w in range(WPC):
            slo, dlo, qlo, shi, dhi, qhi = seg[c, w]
            for (ss, dd, qq, cpw, base) in (
                (slo, dlo, qlo, int(cpws_lo[w]), 0),
                (shi - HALF, dhi, qhi, int(cpws_hi[w]), HALF),
            ):
                if cpw == 0:
                    continue
                n = cpw * P
                k = len(ss)
                fi = np.zeros(n, np.int16)
                fd = np.zeros(n, np.float32)
                fq = np.zeros(n, np.float32)
                fi[:k] = ss.astype(np.int16)
                fd[:k] = (dd - c * NPC) % P
                fq[:k] = qq
                # idx block: [16g + i%16, i//16] = fi[i], replicated g=0..7
                blk = fi.reshape(n // 16, 16).T          # [16, n/16]
                idxP[:, 8 * ci : 8 * ci + n // 16] = np.tile(blk, (8, 1))
                rows = np.arange(n) % P
                cols = (ci + np.arange(n) // P) * P + fd.astype(np.int64)
                Splane[rows, cols] = fq
                ci += cpw
        assert ci == TC
        per_core.append({
            "idxP": idxP,
            "Splane": Splane.astype(BF),
        })
    return [int(v) for v in cpws_lo], [int(v) for v in cpws_hi], TC, per_core


def _build_program(cpws_lo, cpws_hi, TC):
    nc = bacc.Bacc("TRN2", target_bir_lowering=False, debug=False,
                   num_devices=NCORES)

    x_d = nc.dram_tensor("x", [N_NODES, D], BF16, kind="ExternalInput")
    idxP_d = nc.dram_tensor("idxP", [P, 8 * TC], I16, kind="ExternalInput")
    S_d = nc.dram_tensor("S", [P, TC * P], BF16, kind="ExternalInput")
    W1_d = nc.dram_tensor("W1", [D, H], BF16, kind="ExternalInput")
    W2s_d = nc.dram_tensor("W2s", [P, H], BF16, kind="ExternalInput")
    We1_d = nc.dram_tensor("We1", [P, H], BF16, kind="ExternalInput")
    We2s_d = nc.dram_tensor("We2s", [P, P], BF16, kind="ExternalInput")
    b1T_d = nc.dram_tensor("b1T", [P, 2], F32, kind="ExternalInput")
    be1T_d = nc.dram_tensor("be1T", [P, 2], F32, kind="ExternalInput")
    be2d_d = nc.dram_tensor("be2d", [P, 1], F32, kind="ExternalInput")
    xeP_d = nc.dram_tensor("xeP", [P, EMLP_COLS], BF16, kind="ExternalInput")

    xrec_d = nc.dram_tensor("xrec", [NPC, D], F32, kind="ExternalOutput")
    erecP_d = nc.dram_tensor("erecP", [P, EMLP_COLS], F32, kind="ExternalOutput")

    p_loc_d = nc.dram_tensor("p_loc", [NPC, D], BF16)
    p_full_d = nc.dram_tensor("p_full", [N_NODES, D], BF16)

    with tile.TileContext(nc) as tc:
        with (
            tc.tile_pool(name="meta", bufs=1) as meta_tp,
            tc.tile_pool(name="msg", bufs=3) as msg_tp,
            tc.tile_pool(name="sel", bufs=6) as sel_tp,
            tc.tile_pool(name="wsb", bufs=2) as wsb_tp,
            tc.tile_pool(name="emlp", bufs=3) as emlp_tp,
            tc.tile_pool(name="agg_ps", bufs=2, space="PSUM") as aggps_tp,
            tc.tile_pool(name="h_ps", bufs=2, space="PSUM") as hps_tp,
            tc.tile_pool(name="p_ps", bufs=1, space="PSUM") as pps_tp,
            tc.tile_pool(name="e1_ps", bufs=2, space="PSUM") as e1ps_tp,
            tc.tile_pool(name="er_ps", bufs=1, space="PSUM") as erps_tp,
        ):
            idxP = meta_tp.tile([P, 8 * TC], I16)
            W1 = meta_tp.tile([D, H], BF16)
            W2s = meta_tp.tile([P, H], BF16)
            We1 = meta_tp.tile([P, H], BF16)
            We2s = meta_tp.tile([P, P], BF16)
            b1T = meta_tp.tile([P, 2], F32)
            be1T = meta_tp.tile([P, 2], F32)
            be2d = meta_tp.tile([P, 1], F32)
            for sb, dr in ((idxP, idxP_d), (W1, W1_d), (W2s, W2s_d),
                           (We1, We1_d), (We2s, We2s_d), (b1T, b1T_d),
                           (be1T, be1T_d), (be2d, be2d_d)):
                nc.sync.dma_start(sb[:], dr.ap())

            def agg_window(w, ci0, table_d, out_is_dstmajor):
                """One 128-dst window: 1-2 dma_gathers + chunk matmuls."""
                cl, ch = cpws_lo[w], cpws_hi[w]
                cpw = cl + ch
                ps = aggps_tp.tile([P, P], F32, space="PSUM", tag="agg")
                msgw = msg_tp.tile([P, cpw, D], BF16, tag="msg")
                GMAX = 8   # dma_gather fails above 1024 indices
                for (coff0, cn0, row0) in ((0, cl, 0), (cl, ch, HALF)):
                    for sub in range(0, cn0, GMAX):
                        coff = coff0 + sub
                        cn = min(GMAX, cn0 - sub)
                        nidx = cn * P
                        nc.gpsimd.dma_gather(
                            out_ap=msgw[:, coff : coff + cn, :],
                            in_ap=table_d.ap()[row0:, :],
                            idxs_ap=idxP[:, 8 * (ci0 + coff) : 8 * (ci0 + coff) + nidx // 16],
                            num_idxs=nidx,
                            num_idxs_reg=nidx,
                            elem_size=D,
                        )
                Sw = sel_tp.tile([P, cpw, P], BF16, tag="S")
                nc.sync.dma_start(
                    Sw[:].rearrange("p c d -> p (c d)"),
                    S_d.ap()[:, ci0 * P : (ci0 + cpw) * P])
                for j in range(cpw):
                    if out_is_dstmajor:
                        nc.tensor.matmul(out=ps[:], lhsT=Sw[:, j, :],
                                         rhs=msgw[:, j, :],
                                         start=(j == 0), stop=(j == cpw - 1))
                    else:
                        nc.tensor.matmul(out=ps[:], lhsT=msgw[:, j, :],
                                         rhs=Sw[:, j, :],
                                         start=(j == 0), stop=(j == cpw - 1))
                return ps

            def layer1_window(w, ci0):
                agg_ps = agg_window(w, ci0, x_d, out_is_dstmajor=False)
                aggT = wsb_tp.tile([P, P], BF16, tag="aggT")
                nc.scalar.activation(out=aggT[:], in_=agg_ps[:],
                                     func=mybir.ActivationFunctionType.Copy)
                h0ps = hps_tp.tile([P, P], F32, space="PSUM", tag="hps")
                h1ps = hps_tp.tile([P, P], F32, space="PSUM", tag="hps")
                nc.tensor.matmul(out=h0ps[:], lhsT=W1[:, 0:P], rhs=aggT[:],
                                 start=True, stop=True)
                nc.tensor.matmul(out=h1ps[:], lhsT=W1[:, P:H], rhs=aggT[:],
                                 start=True, stop=True)
                h0 = wsb_tp.tile([P, P], BF16, tag="h0")
                h1 = wsb_tp.tile([P, P], BF16, tag="h1")
                nc.scalar.activation(out=h0[:], in_=h0ps[:],
                                     func=mybir.ActivationFunctionType.Relu,
                                     bias=b1T[:, 0:1])
                nc.scalar.activation(out=h1[:], in_=h1ps[:],
                                     func=mybir.ActivationFunctionType.Relu,
                                     bias=b1T[:, 1:2])
                pps = pps_tp.tile([P, P], F32, space="PSUM", tag="pps")
                nc.tensor.matmul(out=pps[:], lhsT=h0[:], rhs=W2s[:, 0:P],
                                 start=True, stop=False)
                nc.tensor.matmul(out=pps[:], lhsT=h1[:], rhs=W2s[:, P:H],
                                 start=False, stop=True)
                psb = wsb_tp.tile([P, P], BF16, tag="psb")
                nc.scalar.activation(out=psb[:], in_=pps[:],
                                     func=mybir.ActivationFunctionType.Copy)
                rows = min(P, NPC - w * P)
                nc.sync.dma_start(p_loc_d.ap()[w * P : w * P + rows, :],
                                  psb[0:rows, :])

            def layer2_window(w, ci0):
                x_ps = agg_window(w, ci0, p_full_d, out_is_dstmajor=True)
                xr = wsb_tp.tile([P, P], F32, tag="xr")
                nc.scalar.activation(out=xr[:], in_=x_ps[:],
                                     func=mybir.ActivationFunctionType.Copy)
                rows = min(P, NPC - w * P)
                nc.sync.dma_start(xrec_d.ap()[w * P : w * P + rows, :],
                                  xr[0:rows, :])

            def emlp_tile(t):
                col0 = t * E_TILE
                xt = emlp_tp.tile([P, E_TILE], BF16, tag="xt")
                nc.sync.dma_start(xt[:], xeP_d.ap()[:, col0 : col0 + E_TILE])
                er_ps = erps_tp.tile([P, E_TILE], F32, space="PSUM", tag="erps")
                for a in (0, 1):
                    e1sb = []
                    for hh in (0, 1):
                        e1ps = e1ps_tp.tile([P, E_TILE], F32, space="PSUM",
                                            tag="e1ps")
                        nc.tensor.matmul(
                            out=e1ps[:],
                            lhsT=We1[a * DE : (a + 1) * DE, hh * P : (hh + 1) * P],
                            rhs=xt[a * DE : (a + 1) * DE, :],
                            start=True, stop=True,
                        )
                        sb = emlp_tp.tile([P, E_TILE], BF16, tag="e1sb")
                        nc.scalar.activation(
                            out=sb[:], in_=e1ps[:],
                            func=mybir.ActivationFunctionType.Relu,
                            bias=be1T[:, hh : hh + 1],
                        )
                        e1sb.append(sb)
                    nc.tensor.matmul(
                        out=er_ps[a * DE : (a + 1) * DE, :],
                        lhsT=We2s[:, 0:DE],
                        rhs=e1sb[0][:],
                        start=True, stop=False,
                        tile_position=(0, a * DE),
                    )
                    nc.tensor.matmul(
                        out=er_ps[a * DE : (a + 1) * DE, :],
                        lhsT=We2s[:, DE : 2 * DE],
                        rhs=e1sb[1][:],
                        start=False, stop=True,
                        tile_position=(0, a * DE),
                    )
                er = emlp_tp.tile([P, E_TILE], F32, tag="er")
                nc.scalar.activation(out=er[:], in_=er_ps[:],
                                     func=mybir.ActivationFunctionType.Identity,
                                     bias=be2d[:])
                nc.sync.dma_start(erecP_d.ap()[:, col0 : col0 + E_TILE], er[:])

            # ---- phase A: layer-1 windows interleaved with edge-MLP tiles
            ci0 = 0
            for w in range(WPC):
                layer1_window(w, ci0)
                ci0 += cpws_lo[w] + cpws_hi[w]
                if w < EMLP_TILES // 2:
                    emlp_tile(w)

            # ---- AllGather p (bf16)
            nc.gpsimd.collective_compute(
                "AllGather",
                mybir.AluOpType.bypass,
                replica_groups=[list(range(NCORES))],
                ins=[p_loc_d.ap()],
                outs=[p_full_d.ap()],
            )

            # ---- phase C: layer-2 windows + remaining eMLP
            ci0 = 0
            for w in range(WPC):
                layer2_window(w, ci0)
                ci0 += cpws_lo[w] + cpws_hi[w]
                if EMLP_TILES // 2 + w < EMLP_TILES:
                    emlp_tile(EMLP_TILES // 2 + w)
            for t in range(EMLP_TILES // 2 + WPC, EMLP_TILES):
                emlp_tile(t)

    nc.compile()
    return nc


_CACHE = {}


def _get_program(cpws_lo, cpws_hi, TC):
    key = (tuple(cpws_lo), tuple(cpws_hi))
    if key not in _CACHE:
        _CACHE[key] = _build_program(cpws_lo, cpws_hi, TC)
    return _CACHE[key]


def kernel(x, edge_index, xe, W1, b1, W2, b2, We1, be1, We2, be2, **run_kwargs):
    out, _ = kernel_run(x, edge_index, xe, W1, b1, W2, b2, We1, be1, We2, be2,
                        **run_kwargs)
    return out


def kernel_run(x, edge_index, xe, W1, b1, W2, b2, We1, be1, We2, be2,
               **run_kwargs):
    x = np.asarray(x, dtype=np.float32)
    xe = np.asarray(xe, dtype=np.float32)
    W1 = np.asarray(W1, dtype=np.float32)
    W2 = np.asarray(W2, dtype=np.float32)
    We1 = np.asarray(We1, dtype=np.float32)
    We2 = np.asarray(We2, dtype=np.float32)
    b1 = np.asarray(b1, dtype=np.float32)
    b2 = np.asarray(b2, dtype=np.float32)
    be1 = np.asarray(be1, dtype=np.float32)
    be2 = np.asarray(be2, dtype=np.float32)

    cpws_lo, cpws_hi, TC, per_core = _preprocess(edge_index)
    nc = _get_program(cpws_lo, cpws_hi, TC)

    x_bf = x.astype(BF)
    We1d = np.concatenate([We1, We1], axis=0).astype(BF)     # [128, 256]
    W2s = np.concatenate([W2[:P], W2[P:]], axis=1).astype(BF)
    We2s = np.concatenate([We2[:P], We2[P:]], axis=1).astype(BF)
    W1b = W1.astype(BF)
    b1T = np.ascontiguousarray(b1.reshape(2, P).T)
    be1T = np.ascontiguousarray(be1.reshape(2, P).T)
    be2d = np.concatenate([be2, be2]).reshape(P, 1)

    in_maps = []
    for c in range(NCORES):
        pc = per_core[c]
        xe_c = xe[c * EPC : (c + 1) * EPC]
        xeP = np.zeros((P, EMLP_COLS), dtype=BF)
        xeP[:, : EPC // 2] = (
            xe_c.reshape(EPC // 2, 2, DE).transpose(1, 2, 0).reshape(P, EPC // 2)
        ).astype(BF)
        in_maps.append({
            "x": x_bf, "idxP": pc["idxP"], "S": pc["Splane"],
            "W1": W1b, "W2s": W2s, "We1": We1d, "We2s": We2s,
            "b1T": b1T, "be1T": be1T, "be2d": be2d, "xeP": xeP,
        })

    res = run_bass_kernel_spmd(nc, in_maps, core_ids=list(range(NCORES)),
                               **run_kwargs)

    x_rec = np.concatenate([res.results[c]["xrec"] for c in range(NCORES)], axis=0)
    x_rec = x_rec + b2[None, :]
    e_rec = np.concatenate(
        [
            res.results[c]["erecP"]
            .reshape(2, DE, EMLP_COLS)[:, :, : EPC // 2]
            .transpose(2, 0, 1)
            .reshape(EPC, DE)
            for c in range(NCORES)
        ],
        axis=0,
    )
    return (x_rec.astype(np.float32), e_rec.astype(np.float32)), res
